# revision 5
# baseline (speedup 1.0000x reference)
"""Trainium2 Bass kernel for nn_Classifier_72258529788341.

Computes, for two ragged batches of sequences x:(16,2048,512) and
y:(16,2048,512) with padding masks, the per-sample max over the valid
prefix [0, len_b) of each sequence, concatenates the two pooled vectors
and applies a (1, 1024) linear layer -> (16, 1) float32.

Strategy (8 NeuronCores, data-parallel over batch, 2 samples/core):
  - Host: lengths len_b = S - mask.sum() are tiny to compute; slice the
    sequence dim to n_t*128 (n_t = ceil(max_len/128)) so the device never
    touches data beyond the longest valid prefix; convert the streamed
    data to fp16 (inputs are ~N(0,1); quantization error ~5e-4 relative).
  - Device: seq positions on SBUF partitions, D on the free dim.
    acc[128, 512] accumulates an elementwise running max over seq tiles:
      * tiles fully inside every sample's prefix: tensor_tensor(max)
        (fp16 SBUF 2x DVE mode)
      * boundary tiles: scalar_tensor_tensor acc=(tile+bias) max acc with
        a per-partition bias column (0 for valid rows, -60000 otherwise)
    Then TensorE transposes the 128x128 blocks of acc into PSUM, one DVE
    reduce_max collapses the 128 seq lanes, a fused multiply+reduce with
    the pre-transposed W gives per-partition partial dots, and a 128-long
    ones matmul does the final cross-partition sum. b added via ScalarE.
"""

import math
import os

import numpy as np

import concourse.bacc as bacc
import concourse.bass as bass
import concourse.mybir as mybir
import concourse.tile as tile
from concourse import masks
from concourse.bass_utils import run_bass_kernel_spmd

B, S, D = 16, 2048, 512
N_CORES = 8
BPC = B // N_CORES  # samples per core
P = 128
NEG = -60000.0  # acts as -inf, representable in fp16
G = 3  # seq tiles per DMA transfer

FP16 = mybir.dt.float16
F32 = mybir.dt.float32

_program_cache: dict[tuple[int, int], bass.Bass] = {}

LAST_RESULTS = None  # BassKernelResults of the most recent run (for test.py)


def _build(n_t: int, t_full: int) -> bass.Bass:
    """Build the per-core SPMD program for n_t seq tiles per sample, of
    which tiles [0, t_full) are valid for every sample (no masking)."""
    Sp = n_t * P
    # Bacc (not plain Bass): its finalize runs generate_event_semaphores,
    # which splits multi-semaphore waits — the TT ISA encoding allows one.
    nc = bacc.Bacc(None)

    x_h = nc.declare_dram_parameter("x", [BPC, Sp, D], FP16, isOutput=False)
    y_h = nc.declare_dram_parameter("y", [BPC, Sp, D], FP16, isOutput=False)
    bx_h = nc.declare_dram_parameter("bx", [BPC, P, n_t], FP16, isOutput=False)
    by_h = nc.declare_dram_parameter("by", [BPC, P, n_t], FP16, isOutput=False)
    wt_h = nc.declare_dram_parameter("wt", [P, 2 * D // P], F32, isOutput=False)
    bb_h = nc.declare_dram_parameter("bb", [BPC, 1], F32, isOutput=False)
    out_h = nc.declare_dram_parameter("out", [BPC, 1], F32, isOutput=True)

    n_blk = 2 * D // P  # 8 transpose blocks per sample (x: 0-3, y: 4-7)
    units = []  # (unit_idx, sample, src dram, bias dram)
    for sb in range(BPC):
        units.append((2 * sb + 0, sb, x_h, bx_h))
        units.append((2 * sb + 1, sb, y_h, by_h))

    with tile.TileContext(nc) as tc:
        with (
            tc.tile_pool(name="const", bufs=1) as const_pool,
            tc.tile_pool(name="acc", bufs=1) as acc_pool,
            tc.tile_pool(name="data", bufs=8) as data_pool,
            tc.tile_pool(name="fin", bufs=1) as fin_pool,
            tc.tile_pool(name="psum", bufs=1, space="PSUM") as psum_pool,
        ):
            ident = const_pool.tile([P, P], FP16, tag="ident")
            masks.make_identity(nc, ident[:])
            wt = const_pool.tile([P, n_blk], F32, tag="wt")
            nc.sync.dma_start(out=wt[:], in_=wt_h[:])
            bb = const_pool.tile([BPC, 1], F32, tag="bb")
            nc.sync.dma_start(out=bb[:], in_=bb_h[:])
            ones = const_pool.tile([P, 1], F32, tag="ones")
            nc.gpsimd.memset(ones[:], 1.0)

            bias_tiles = {}
            accs = {}
            for u, sb, src, bias in units:
                if t_full < n_t:
                    bt = const_pool.tile([P, n_t], FP16, tag=f"bias{u}")
                    nc.sync.dma_start(out=bt[:], in_=bias[sb])
                    bias_tiles[u] = bt
                acc = acc_pool.tile([P, D], FP16, tag=f"acc{u}")
                nc.gpsimd.memset(acc[:], NEG)
                accs[u] = acc

            # Streaming accumulation, interleaved across the 4 units so the
            # DVE always has an independent chain to work on.
            n_groups = math.ceil(n_t / G)
            for g in range(n_groups):
                t0, t1 = g * G, min((g + 1) * G, n_t)
                gsz = t1 - t0
                for u, sb, src, bias in units:
                    dt = data_pool.tile([P, gsz * D], FP16, tag="data")
                    src_ap = src[sb, t0 * P : t1 * P, :].rearrange(
                        "(t p) d -> p t d", p=P
                    )
                    nc.sync.dma_start(
                        out=dt[:].rearrange("p (t d) -> p t d", d=D), in_=src_ap
                    )
                    acc = accs[u]
                    for t in range(t0, t1):
                        dslice = dt[:, (t - t0) * D : (t - t0 + 1) * D]
                        if t < t_full:
                            nc.vector.tensor_tensor(
                                acc[:], dslice, acc[:], op=mybir.AluOpType.max
                            )
                        else:
                            nc.vector.scalar_tensor_tensor(
                                acc[:],
                                dslice,
                                bias_tiles[u][:, t : t + 1],
                                acc[:],
                                op0=mybir.AluOpType.add,
                                op1=mybir.AluOpType.max,
                            )

            # Finalization: per sample, transpose the accumulated maxes so D
            # lands on partitions, collapse the 128 seq lanes, dot with W.
            dvec = fin_pool.tile([P, BPC], F32, tag="dvec")
            for sb in range(BPC):
                pt = psum_pool.tile([P, n_blk * P], FP16, tag=f"tr{sb}")
                for half in range(2):  # 0: x acc, 1: y acc
                    acc = accs[2 * sb + half]
                    for k in range(D // P):
                        blk = half * (D // P) + k
                        nc.tensor.transpose(
                            pt[:, blk * P : (blk + 1) * P],
                            acc[:, k * P : (k + 1) * P],
                            ident[:],
                        )
                m = fin_pool.tile([P, n_blk], F32, tag=f"m{sb}")
                nc.vector.reduce_max(
                    m[:],
                    pt[:].rearrange("p (k s) -> p k s", k=n_blk),
                    axis=mybir.AxisListType.X,
                )
                # NOTE: tensor_tensor_reduce wedges the device at execution
                # (NRT unrecoverable) — use separate mult + reduce_sum.
                scratch = fin_pool.tile([P, n_blk], F32, tag=f"sc{sb}")
                nc.vector.tensor_tensor(
                    scratch[:], m[:], wt[:], op=mybir.AluOpType.mult
                )
                nc.vector.reduce_sum(
                    dvec[:, sb : sb + 1], scratch[:], axis=mybir.AxisListType.X
                )
            psum_out = psum_pool.tile([BPC, 1], F32, tag="out")
            nc.tensor.matmul(psum_out[:], dvec[:], ones[:], start=True, stop=True)
            out_sb = fin_pool.tile([BPC, 1], F32, tag="osb")
            nc.scalar.activation(
                out_sb[:],
                psum_out[:],
                mybir.ActivationFunctionType.Identity,
                bias=bb[:],
                scale=1.0,
            )
            nc.sync.dma_start(out=out_h[:], in_=out_sb[:])

    # run_bass_via_pjrt binds the exec primitive without finalizing; Bacc's
    # finalize runs the compile passes (reg alloc, event-semaphore splitting).
    nc.finalize()
    return nc


def kernel(x, y, mask_x, mask_y, W, b):
    global LAST_RESULTS
    x = np.asarray(x)
    y = np.asarray(y)
    mask_x = np.asarray(mask_x)
    mask_y = np.asarray(mask_y)
    W = np.asarray(W, dtype=np.float32)
    b = np.asarray(b, dtype=np.float32)

    len_x = (x.shape[1] - mask_x.sum(axis=1)).astype(np.int64)  # (B,)
    len_y = (y.shape[1] - mask_y.sum(axis=1)).astype(np.int64)
    max_len = int(max(len_x.max(), len_y.max(), 1))
    min_len = int(min(len_x.min(), len_y.min()))
    n_t = math.ceil(max_len / P)
    t_full = min_len // P
    Sp = n_t * P

    xs = np.ascontiguousarray(x[:, :Sp, :], dtype=np.float16)
    ys = np.ascontiguousarray(y[:, :Sp, :], dtype=np.float16)
    pos = np.arange(n_t)[None, :] * P + np.arange(P)[:, None]  # (P, n_t)
    bx = np.where(pos[None] < len_x[:, None, None], 0.0, NEG).astype(np.float16)
    by = np.where(pos[None] < len_y[:, None, None], 0.0, NEG).astype(np.float16)
    wt = np.ascontiguousarray(W[0].reshape(2 * D // P, P).T, dtype=np.float32)
    bb = np.full((BPC, 1), float(b[0]), dtype=np.float32)

    key = (n_t, t_full)
    if key not in _program_cache:
        _program_cache[key] = _build(n_t, t_full)
    nc = _program_cache[key]

    in_maps = []
    for c in range(N_CORES):
        sl = slice(c * BPC, (c + 1) * BPC)
        in_maps.append(
            {
                "x": xs[sl],
                "y": ys[sl],
                "bx": np.ascontiguousarray(bx[sl]),
                "by": np.ascontiguousarray(by[sl]),
                "wt": wt,
                "bb": bb,
            }
        )

    res = run_bass_kernel_spmd(nc, in_maps, list(range(N_CORES)))
    LAST_RESULTS = res
    out = np.concatenate([res.results[c]["out"] for c in range(N_CORES)], axis=0)
    return out.astype(np.float32)


# revision 7
# speedup vs baseline: 1.1145x; 1.1145x over previous
"""Trainium2 Bass kernel for nn_Classifier_72258529788341.

Computes, for two ragged batches of sequences x:(16,2048,512) and
y:(16,2048,512) with padding masks, the per-sample max over the valid
prefix [0, len_b) of each sequence, concatenates the two pooled vectors
and applies a (1, 1024) linear layer -> (16, 1) float32.

Strategy (8 NeuronCores, data-parallel over batch, 2 samples/core):
  - Host (inspector): lengths len_b = S - mask.sum() are trivial; slice
    the sequence dim to n_t*128 (n_t = ceil(max_len/128)) so the device
    never reads past the longest prefix, convert the streamed data to
    fp16 (inputs ~N(0,1); quantization ~5e-4 relative), and overwrite the
    padded tail rows [len_b, n_t*128) with -60000 so the device kernel is
    a pure max-reduction with no masking.
  - Device (executor): seq positions on SBUF partitions, D on free dim.
    Each (sample, tensor) unit streams its tiles in two DMA groups and
    tree-folds them with wide fp16 tensor_tensor(max) ops (2x DVE mode)
    into acc[128, 512].  TensorE then transposes the 128x128 blocks of
    acc into PSUM, DVE reduce_max collapses the 128 seq lanes, a small
    dot with the pre-transposed W and a 128-ones matmul finish the
    linear layer; ScalarE adds the bias b.
"""

import math

import numpy as np

import concourse.bacc as bacc
import concourse.mybir as mybir
import concourse.tile as tile
from concourse import masks
from concourse.bass_utils import run_bass_kernel_spmd

B, S, D = 16, 2048, 512
N_CORES = 8
BPC = B // N_CORES  # samples per core
P = 128
NEG = -60000.0  # acts as -inf, representable in fp16

FP16 = mybir.dt.float16
F32 = mybir.dt.float32

_program_cache: dict[int, object] = {}

LAST_RESULTS = None  # BassKernelResults of the most recent run (for test.py)


def _build(n_t: int):
    """Per-core SPMD program: n_t seq tiles per sample, data pre-masked."""
    Sp = n_t * P
    nc = bacc.Bacc(None)

    x_h = nc.declare_dram_parameter("x", [BPC, Sp, D], FP16, isOutput=False)
    y_h = nc.declare_dram_parameter("y", [BPC, Sp, D], FP16, isOutput=False)
    wt_h = nc.declare_dram_parameter("wt", [P, 2 * D // P], F32, isOutput=False)
    bb_h = nc.declare_dram_parameter("bb", [BPC, 1], F32, isOutput=False)
    out_h = nc.declare_dram_parameter("out", [BPC, 1], F32, isOutput=True)

    n_blk = 2 * D // P  # 8 transpose blocks per sample (x: 0-3, y: 4-7)
    # two DMA groups per unit so compute starts after ~half the transfer
    g0 = (n_t + 1) // 2
    groups = [(0, g0)] + ([(g0, n_t)] if n_t > g0 else [])

    def fold_group(dt, gsz, acc, acc_written):
        """Tree-fold gsz seq tiles living side by side in dt into acc."""
        F = gsz
        while F > (1 if acc_written else 2):
            h = F // 2
            nc.vector.tensor_tensor(
                dt[:, : h * D],
                dt[:, : h * D],
                dt[:, (F - h) * D : F * D],
                op=mybir.AluOpType.max,
            )
            F -= h
        if acc_written:
            nc.vector.tensor_tensor(
                acc[:], dt[:, :D], acc[:], op=mybir.AluOpType.max
            )
        elif F == 2:
            nc.vector.tensor_tensor(
                acc[:], dt[:, :D], dt[:, D : 2 * D], op=mybir.AluOpType.max
            )
        else:
            nc.vector.tensor_copy(acc[:], dt[:, :D])
        return True

    with tile.TileContext(nc) as tc:
        with (
            tc.tile_pool(name="const", bufs=1) as const_pool,
            tc.tile_pool(name="acc", bufs=1) as acc_pool,
            tc.tile_pool(name="data", bufs=6) as data_pool,
            tc.tile_pool(name="fin", bufs=1) as fin_pool,
            tc.tile_pool(name="psum", bufs=1, space="PSUM") as psum_pool,
        ):
            # constants: keep the SP HWDGE ring free for data; use the ACT
            # ring (nc.scalar.dma_start) and GpSimd for everything else.
            ident = const_pool.tile([P, P], FP16, tag="ident")
            masks.make_identity(nc, ident[:])
            wt = const_pool.tile([P, n_blk], F32, tag="wt")
            nc.scalar.dma_start(out=wt[:], in_=wt_h[:])
            bb = const_pool.tile([BPC, 1], F32, tag="bb")
            nc.scalar.dma_start(out=bb[:], in_=bb_h[:])
            ones = const_pool.tile([P, 1], F32, tag="ones")
            nc.gpsimd.memset(ones[:], 1.0)

            dvec = fin_pool.tile([P, BPC], F32, tag="dvec")
            pts = {}
            ms = {}
            for sb in range(BPC):
                pts[sb] = psum_pool.tile(
                    [P, n_blk * P], FP16, tag=f"tr{sb}", name=f"tr{sb}"
                )
                ms[sb] = fin_pool.tile(
                    [P, n_blk], F32, tag=f"m{sb}", name=f"m{sb}"
                )

            def stream_unit(sb, src, tag):
                acc = acc_pool.tile([P, D], FP16, tag=f"acc{tag}")
                written = False
                for t0, t1 in groups:
                    gsz = t1 - t0
                    dt = data_pool.tile([P, gsz * D], FP16, tag="data")
                    nc.sync.dma_start(
                        out=dt[:].rearrange("p (t d) -> p t d", d=D),
                        in_=src[sb, t0 * P : t1 * P, :].rearrange(
                            "(t p) d -> p t d", p=P
                        ),
                    )
                    written = fold_group(dt, gsz, acc, written)
                return acc

            def finalize_unit(sb, half, acc):
                pt = pts[sb]
                for k in range(D // P):
                    blk = half * (D // P) + k
                    nc.tensor.transpose(
                        pt[:, blk * P : (blk + 1) * P],
                        acc[:, k * P : (k + 1) * P],
                        ident[:],
                    )
                nc.vector.reduce_max(
                    ms[sb][:, half * 4 : half * 4 + 4],
                    pt[:, half * D : (half + 1) * D].rearrange(
                        "p (k s) -> p k s", k=D // P
                    ),
                    axis=mybir.AxisListType.X,
                )

            for sb in range(BPC):
                acc_x = stream_unit(sb, x_h, f"{sb}x")
                acc_y = stream_unit(sb, y_h, f"{sb}y")
                finalize_unit(sb, 0, acc_x)
                finalize_unit(sb, 1, acc_y)
                scratch = fin_pool.tile([P, n_blk], F32, tag=f"sc{sb}")
                nc.vector.tensor_tensor(
                    scratch[:], ms[sb][:], wt[:], op=mybir.AluOpType.mult
                )
                nc.vector.reduce_sum(
                    dvec[:, sb : sb + 1], scratch[:], axis=mybir.AxisListType.X
                )

            psum_out = psum_pool.tile([BPC, 1], F32, tag="out")
            nc.tensor.matmul(psum_out[:], dvec[:], ones[:], start=True, stop=True)
            out_sb = fin_pool.tile([BPC, 1], F32, tag="osb")
            nc.scalar.activation(
                out_sb[:],
                psum_out[:],
                mybir.ActivationFunctionType.Identity,
                bias=bb[:],
                scale=1.0,
            )
            nc.scalar.dma_start(out=out_h[:], in_=out_sb[:])

    # run_bass_via_pjrt binds the exec primitive without finalizing; Bacc's
    # finalize runs the compile passes (reg alloc, event-semaphore splitting).
    nc.finalize()
    return nc


def kernel(x, y, mask_x, mask_y, W, b):
    global LAST_RESULTS
    x = np.asarray(x)
    y = np.asarray(y)
    mask_x = np.asarray(mask_x)
    mask_y = np.asarray(mask_y)
    W = np.asarray(W, dtype=np.float32)
    b = np.asarray(b, dtype=np.float32)

    len_x = (x.shape[1] - mask_x.sum(axis=1)).astype(np.int64)  # (B,)
    len_y = (y.shape[1] - mask_y.sum(axis=1)).astype(np.int64)
    max_len = int(max(len_x.max(), len_y.max(), 1))
    n_t = math.ceil(max_len / P)
    Sp = n_t * P

    xs = np.ascontiguousarray(x[:, :Sp, :], dtype=np.float16)
    ys = np.ascontiguousarray(y[:, :Sp, :], dtype=np.float16)
    pad = np.arange(Sp)[None, :]
    xs[pad >= len_x[:, None]] = np.float16(NEG)
    ys[pad >= len_y[:, None]] = np.float16(NEG)
    wt = np.ascontiguousarray(W[0].reshape(2 * D // P, P).T, dtype=np.float32)
    bb = np.full((BPC, 1), float(b[0]), dtype=np.float32)

    if n_t not in _program_cache:
        _program_cache[n_t] = _build(n_t)
    nc = _program_cache[n_t]

    in_maps = []
    for c in range(N_CORES):
        sl = slice(c * BPC, (c + 1) * BPC)
        in_maps.append({"x": xs[sl], "y": ys[sl], "wt": wt, "bb": bb})

    res = run_bass_kernel_spmd(nc, in_maps, list(range(N_CORES)))
    LAST_RESULTS = res
    out = np.concatenate([res.results[c]["out"] for c in range(N_CORES)], axis=0)
    return out.astype(np.float32)
